# revision 77
# baseline (speedup 1.0000x reference)
"""CondConv2D Trainium2 kernel: data-parallel over batch across 8 NeuronCores.

Per core (4 samples):
  1. alphas = softmax(cond @ alpha_w + alpha_b)              [tiny PE matmul + ACT/DVE]
  2. W_mix[b] = sum_e alphas[b,e] * expert_kernels[e]        [ACT mul + DVE scalar_tensor_tensor]
  3. conv2d(x[b], W_mix[b], SAME) + bias_mix[b]

Conv strategy (output-stationary, bf16-view PE transposes):
  x[b] loads in natural [h, (w,c)] fp32 layout; per 2-column block m a PE
  transpose reads the bf16 HIGH HALVES of the fp32 elements (stride-2
  bitcast view, 1 cyc/row vs fp32's 2) against a bf16 identity and lands
  XT[(delta,c), slot m+1, 1:129] via bf16 PSUM + ACT copy, where delta =
  column parity within the block. PSUM holds out[h, 4cols, F]; for
  each output column w (block m=w>>1) 6 matmuls accumulate with XT slices as
  the STATIONARY operand (M = h window) and mixed weights as the MOVING
  operand (N = F): 3 full-K matmuls cover the tap pair (kw in {1,2} for even
  w via w12, {0,1} for odd w via w01) per kh, and 3 half-K matmuls cover the
  leftover column (w-1 for even w from XT[64:], w+1 for odd w from XT[:64])
  via wodd (lower half = kw2 weights, upper half = kw0). Output needs no PE
  transpose: DVE adds the broadcast mixed bias during PSUM evacuation and the
  store DMA writes contiguous 2KB runs to HBM [B,H,W,F].

Overlap: routing exp-weights stay UNNORMALIZED for the mixes (1/sum rides the
evacuation stt), alpha_b folds into the logits matmul as a 65th ones-row,
alphas/bias broadcast to 128 partitions via one-hot mask matmuls (no DRAM
round-trip), the ACT Exp table is preloaded by a dummy activation, transpose
batches interleave into the conv group stream, sample 0 computes even columns
first (so only ekodd/ek12 gate the first group), and the next sample's x
chunks + weight mixes are issued mid-sample so PE never waits at boundaries.
"""

import numpy as np

import concourse.bass as bass
import concourse.bacc as bacc
import concourse.mybir as mybir
import concourse.tile as tile
from concourse.bass_utils import run_bass_kernel_spmd
from concourse.masks import make_identity

B, H, W, Cin, E, F = 32, 128, 128, 64, 4, 128
KH = KW = 3
NCORES = 8
NB = B // NCORES  # 4 samples per core
CD = 64  # cond dim
HP = H + 2  # padded j range: j=0 -> row -1, j=129 -> row 128
NBLK = W // 2  # 64 two-column blocks
NS = NBLK + 2  # slots: 0 = cols (-2,-1), 1..64 = blocks, 65 = cols (128,129)

FP32 = mybir.dt.float32
BF16 = mybir.dt.bfloat16
AF = mybir.ActivationFunctionType
ALU = mybir.AluOpType

_cache = {}


def _build_nc():
    nc = bacc.Bacc(None)
    x_in = nc.dram_tensor("x", [NB, H, W, Cin], FP32, kind="ExternalInput")
    cond_in = nc.dram_tensor("cond", [NB, CD], FP32, kind="ExternalInput")
    aw_in = nc.dram_tensor("alpha_w", [CD, E], FP32, kind="ExternalInput")
    ab_in = nc.dram_tensor("alpha_b", [E], FP32, kind="ExternalInput")
    ek_in = nc.dram_tensor("expert_kernels", [E, KH, KW, Cin, F], FP32, kind="ExternalInput")
    eb_in = nc.dram_tensor("expert_bias", [E, F], FP32, kind="ExternalInput")
    out_t = nc.dram_tensor("out", [NB, H, W, F], FP32, kind="ExternalOutput")

    with tile.TileContext(nc) as tc:
        with (
            tc.tile_pool(name="const", bufs=1) as const_pool,
            tc.tile_pool(name="ek", bufs=1) as ek_pool,
            tc.tile_pool(name="mix", bufs=2) as mix_pool,
            tc.tile_pool(name="wts", bufs=2) as w_pool,
            tc.tile_pool(name="xt", bufs=2) as xt_pool,
            tc.tile_pool(name="bias", bufs=2) as bias_pool,
            tc.tile_pool(name="ev", bufs=8) as ev_pool,
            tc.tile_pool(name="small", bufs=2) as small_pool,
            tc.tile_pool(name="xin", bufs=2) as x_pool,
            tc.tile_pool(name="pconv", bufs=5, space="PSUM") as pconv_pool,
            tc.tile_pool(name="ptin", bufs=2, space="PSUM") as ptin_pool,
            tc.tile_pool(name="psmall", bufs=1, space="PSUM") as psmall_pool,
        ):
            # warm the ACT Exp table while the first DMAs run
            warm = small_pool.tile([1, 4], FP32)
            nc.gpsimd.memset(warm[:, :], 0.0)
            warm2 = small_pool.tile([1, 4], FP32)
            nc.scalar.activation(warm2[:, :], warm[:, :], AF.Exp)

            # ---- tiny routing DMAs first so they aren't starved behind the
            # big ek/x transfers on the shared DMA engines. alpha_b folds into
            # the logits matmul as a 65th contraction row (ones x ab).
            condT = small_pool.tile([CD + 1, NB], FP32)
            nc.sync.dma_start(
                out=condT[0:CD, :],
                in_=bass.AP(tensor=cond_in, offset=0, ap=[[1, CD], [CD, NB]]),
            )
            nc.gpsimd.memset(condT[CD:CD + 1, :], 1.0)
            aw_sb = small_pool.tile([CD + 1, E], FP32)
            nc.sync.dma_start(out=aw_sb[0:CD, :], in_=aw_in[:, :])
            nc.gpsimd.dma_start(
                out=aw_sb[CD:CD + 1, :],
                in_=bass.AP(tensor=ab_in, offset=0, ap=[[0, 1], [1, E]]),
            )
            eb_sb = small_pool.tile([E, F], FP32)
            nc.gpsimd.dma_start(out=eb_sb[:, :], in_=eb_in[:, :])

            # ---- big staging loads: ek first (they gate the weight mixes and
            # hence the first conv group), then x[0] chunks (gate transposes)
            sE, sKH, sKW, sC = KH * KW * Cin * F, KW * Cin * F, Cin * F, F
            NXC = 8  # x chunks per sample
            XCW = W // NXC
            x_tiles = {}

            def stage_x_chunk(b, wq):
                if wq == 0:
                    x_h_new = x_pool.tile([H, W, Cin], FP32, tag="xh")
                    x_tiles[b] = x_h_new
                ws = wq * XCW
                nc.sync.dma_start(out=x_tiles[b][:, ws:ws + XCW, :],
                                  in_=x_in[b, :, ws:ws + XCW, :])

            # load order matches the serial DVE mix chains: ekodd, ek12, ek01
            # ekodd: partitions 0:64 = kw=2 (odd-w leftover), 64:128 = kw=0 (even-w)
            ekodd = ek_pool.tile([128, E, KH, F], FP32)
            nc.sync.dma_start(
                out=ekodd[0:Cin, :, :, :],
                in_=bass.AP(tensor=ek_in, offset=2 * sKW,
                            ap=[[sC, Cin], [sE, E], [sKH, KH], [1, F]]),
            )
            nc.sync.dma_start(
                out=ekodd[Cin:128, :, :, :],
                in_=bass.AP(tensor=ek_in, offset=0,
                            ap=[[sC, Cin], [sE, E], [sKH, KH], [1, F]]),
            )
            # ek12[(d,c), e, kh, f]: d=0 -> kw=1, d=1 -> kw=2  (pairs for even w)
            ek12 = ek_pool.tile([128, E, KH, F], FP32)
            nc.sync.dma_start(
                out=ek12[:, :, :, :],
                in_=bass.AP(tensor=ek_in, offset=sKW,
                            ap=[[sKW, 2], [sC, Cin], [sE, E], [sKH, KH], [1, F]]),
            )
            # all x[0] chunks load before ek01: odd-column groups (the only
            # consumers of w01) don't start until ~30us in, so ek01 has slack
            for wq in range(NXC):
                stage_x_chunk(0, wq)
            # ek01[(d,c), e, kh, f]: d=0 -> kw=0, d=1 -> kw=1  (pairs for odd w)
            ek01 = ek_pool.tile([128, E, KH, F], FP32)
            nc.sync.dma_start(
                out=ek01[:, :, :, :],
                in_=bass.AP(tensor=ek_in, offset=0,
                            ap=[[sKW, 2], [sC, Cin], [sE, E], [sKH, KH], [1, F]]),
            )

            ident = const_pool.tile([128, 128], FP32)
            make_identity(nc, ident[:, :])
            identb = const_pool.tile([128, 128], BF16)
            nc.vector.tensor_copy(identb[:, :], ident[:, :])
            # one-hot masks: mdiag[:, b, :] as K=NB stationary broadcasts row b
            # of a [NB, *] moving tile to all 128 output partitions; built by
            # broadcasting identity columns (partition-base-0 ops only)
            mdiag = const_pool.tile([NB, NB, 128], FP32)
            nc.gpsimd.memset(mdiag[:, :, :], 0.0)
            for b in range(NB):
                nc.vector.tensor_scalar_add(
                    mdiag[:, b, :], mdiag[:, b, :], ident[0:NB, b:b + 1])
            # logits include alpha_b via the ones row; exp is left UNNORMALIZED
            # for the weight mixes — 1/sum(exp) is applied per sample during
            # PSUM evacuation, taking the softmax sum off the critical path
            p_log = psmall_pool.tile([NB, E], FP32, tag="ps")
            nc.tensor.matmul(p_log[:, :], condT[:, :], aw_sb[:, :], start=True, stop=True)
            aexp = small_pool.tile([NB, E], FP32)
            nc.scalar.activation(aexp[:, :], p_log[:, :], AF.Exp)

            # broadcast unnormalized weights to all partitions via mask matmuls
            pa_bc = psmall_pool.tile([128, NB, E], FP32, tag="ps")
            for b in range(NB):
                nc.tensor.matmul(pa_bc[:, b, :], mdiag[:, b, :],
                                 aexp[:, :], start=True, stop=True)
            a_bc = const_pool.tile([128, NB, E], FP32)
            nc.vector.tensor_copy(a_bc[:, :, :], pa_bc[:, :, :])

            # 1/sum(exp) + normalized bias chain: only needed by the first
            # PSUM evacuation (~11us in)
            rec_bc = const_pool.tile([128, NB], FP32)
            bias_bf = small_pool.tile([NB, F], FP32)

            def emit_recip_bias():
                asum = small_pool.tile([NB, 1], FP32)
                nc.vector.reduce_sum(out=asum[:, :], in_=aexp[:, :],
                                     axis=mybir.AxisListType.X)
                arec = small_pool.tile([NB, 1], FP32)
                nc.vector.reciprocal(arec[:, :], asum[:, :])
                prb = psmall_pool.tile([128, NB], FP32, tag="ps")
                for b in range(NB):
                    nc.tensor.matmul(prb[:, b:b + 1], mdiag[:, b, :],
                                     arec[:, :], start=True, stop=True)
                nc.vector.tensor_copy(rec_bc[:, :], prb[:, :])
                # bias_bf[b, f] = (sum_e expert_bias[e,f]*exp[b,e]) / sum(exp[b])
                aT_ps = psmall_pool.tile([E, NB], FP32, tag="ps")
                nc.tensor.transpose(aT_ps[:, :], aexp[:, :], ident[0:E, 0:NB])
                aT_sb = small_pool.tile([E, NB], FP32)
                nc.vector.tensor_copy(aT_sb[:, :], aT_ps[:, :])
                pbias = psmall_pool.tile([F, NB], FP32, tag="ps")
                nc.tensor.matmul(pbias[:, :], eb_sb[:, :], aT_sb[:, :],
                                 start=True, stop=True)
                bias_fb = small_pool.tile([F, NB], FP32)
                nc.vector.tensor_copy(bias_fb[:, :], pbias[:, :])
                pbT = psmall_pool.tile([NB, F], FP32, tag="ps")
                nc.tensor.transpose(pbT[:, :], bias_fb[:, :], ident[0:F, 0:F])
                nc.vector.tensor_scalar_mul(bias_bf[:, :], pbT[:, :], arec[:, 0:1])

            emit_recip_bias()

            # ---- per-sample prep: weight mixes + bias broadcast
            w_tiles = {}

            def prep_steps(b, eng=None, split=False):
                """Return a list of closures, each issuing one mix/setup step;
                callers spread them across conv groups to avoid bursting DVE."""
                eng = eng or nc.vector

                def alpha_ap(e):
                    return a_bc[:, b, e:e + 1]

                def mix_steps(ek_stage, out_tile, tg, meng=None):
                    meng = meng or eng
                    acc = mix_pool.tile([128, KH * F], FP32, tag=tg)

                    def first():
                        nc.scalar.mul(
                            acc[:, :],
                            ek_stage[:, 0, :, :].rearrange("p k f -> p (k f)"),
                            alpha_ap(0))
                    steps = [first]
                    for e in range(1, E):
                        def stt(e=e):
                            src = ek_stage[:, e, :, :].rearrange("p k f -> p (k f)")
                            dst = (acc[:, :] if e < E - 1
                                   else out_tile[:, :, :].rearrange("p k f -> p (k f)"))
                            meng.scalar_tensor_tensor(
                                out=dst, in0=src, scalar=alpha_ap(e), in1=acc[:, :],
                                op0=ALU.mult, op1=ALU.add)
                        steps.append(stt)
                    return steps

                wodd = w_pool.tile([128, KH, F], BF16, tag="wodd")
                w12 = w_pool.tile([128, KH, F], BF16, tag="w12")
                w01 = w_pool.tile([128, KH, F], BF16, tag="w01")
                steps = mix_steps(ekodd, wodd, "acc") + mix_steps(ek12, w12, "acc")
                w01_steps = mix_steps(ek01, w01, "acc")

                def bias_step():
                    pbb = psmall_pool.tile([128, F], FP32, tag="ps")
                    nc.tensor.matmul(pbb[:, :], mdiag[:, b, :],
                                     bias_bf[:, :], start=True, stop=True)
                    bias_b = bias_pool.tile([128, F], FP32)
                    nc.vector.tensor_copy(bias_b[:, :], pbb[:, :])
                    w_tiles[b] = (w01, w12, wodd, bias_b, xt_prep[b])
                steps.append(bias_step)

                def xt_step():
                    # XT[(d,c), slot, j] = x[b, j-1, 2*(slot-1)+d, c]
                    xt = xt_pool.tile([128, NS, HP], BF16)
                    nc.gpsimd.memset(xt[:, 0, :], 0.0)          # cols (-2, -1)
                    nc.gpsimd.memset(xt[:, NS - 1, :], 0.0)     # cols (128, 129)
                    nc.gpsimd.memset(xt[:, 1:NS - 1, 0:1], 0.0)      # row -1
                    nc.gpsimd.memset(xt[:, 1:NS - 1, HP - 1:HP], 0.0)  # row 128
                    xt_prep[b] = xt
                return [xt_step] + steps + w01_steps

            xt_prep = {}

            # sample 0: issue everything except the w01 chain, which waits for
            # ek01 (loaded last) — deferring it keeps the DVE queue clear for
            # the first evacuations; w01 is only read by the odd-column phase
            p0 = prep_steps(0, eng=nc.vector)
            w01_defer = p0[-E:]
            for step in p0[:-E]:
                step()

            def tbatch(b, k):
                ptq = ptin_pool.tile([128, 4, H], BF16, tag="ptin")
                for j4 in range(4):
                    m = 4 * k + j4
                    nc.tensor.matmul(
                        ptq[:, j4, :],
                        xhi_tiles[b][:, 2 * m:2 * m + 2, :].rearrange("h w c -> h (w c)"),
                        identb[:, :], is_transpose=True)
                nc.scalar.copy(xt_tiles[b][:, 4 * k + 1:4 * k + 5, 1:H + 1],
                               ptq[:, :, :])

            xhi_tiles = {}
            xt_tiles = {}

            xhi_tiles[0] = x_tiles[0][:, :, :].bitcast(BF16)[:, :, 1::2]
            xt_tiles[0] = w_tiles[0][4]
            for k in range(3):
                tbatch(0, k)

            for b in range(NB):
                w01, w12, wodd, bias_b, xt = w_tiles.pop(b)
                bt = bias_b[:, :]
                bias_bc4 = bass.AP(tensor=bt.tensor, offset=bt.offset,
                                   ap=[list(bt.ap[0]), [0, 4], [1, F]])
                x_tiles.pop(b)

                # ---- conv: 32 groups of 4 output columns, PSUM [h, 4, F];
                # remaining transpose batches + next-sample prefetch ride along.
                # Sample 0 computes even columns first: they only need the
                # ekodd/ek12 mixes, which load before ek01 and the x chunks.
                if b == 0:
                    glist = ([list(range(8 * g, 8 * g + 8, 2)) for g in range(16)]
                             + [list(range(8 * g + 1, 8 * g + 8, 2)) for g in range(16)])
                else:
                    glist = [list(range(4 * g, 4 * g + 4)) for g in range(32)]
                for g, gcols in enumerate(glist):
                    if g < 13:
                        tbatch(b, 3 + g)
                    if b == 0 and 4 <= g < 4 + len(w01_defer):
                        w01_defer[g - 4]()
                    if b + 1 < NB:
                        if 8 <= g < 8 + 2 * NXC and g % 2 == 0:
                            stage_x_chunk(b + 1, (g - 8) // 2)
                        if g == 14:
                            nsteps = prep_steps(b + 1)
                        # one prep step per group keeps DVE free for evacs
                        if 14 <= g < 14 + len(nsteps):
                            nsteps[g - 14]()
                        if g >= 28 and g == 14 + len(nsteps):
                            xhi_tiles[b + 1] = (
                                x_tiles[b + 1][:, :, :].bitcast(BF16)[:, :, 1::2])
                            xt_tiles[b + 1] = w_tiles[b + 1][4]
                        # first 3 transpose batches of b+1 overlap b's last groups
                        if g >= 29:
                            tbatch(b + 1, g - 29)
                    pc = pconv_pool.tile([H, 4, F], FP32)
                    for j, w in enumerate(gcols):
                        s = (w >> 1) + 1
                        if w % 2 == 0:
                            pair_w, left_lo, left_hi, ls = w12, Cin, 128, s - 1
                        else:
                            pair_w, left_lo, left_hi, ls = w01, 0, Cin, s + 1
                        for kh in range(KH):
                            nc.tensor.matmul(
                                pc[:, j, :],
                                xt[:, s, kh:kh + H],
                                pair_w[:, kh, :],
                                start=(kh == 0), stop=False)
                        for kh in range(KH):
                            nc.tensor.matmul(
                                pc[:, j, :],
                                xt[left_lo:left_hi, ls, kh:kh + H],
                                wodd[left_lo:left_hi, kh, :],
                                start=False, stop=(kh == KH - 1))
                    # evacuate: normalize by 1/sum(exp) and add bias; alternate
                    # DVE/Pool so neither engine gates the PSUM recycle
                    ev = ev_pool.tile([H, 4, F], FP32)
                    w0, w1 = gcols[0], gcols[1]
                    nc.vector.scalar_tensor_tensor(
                        out=ev[:, :, :], in0=pc[:, :, :],
                        scalar=rec_bc[:, b:b + 1], in1=bias_bc4,
                        op0=ALU.mult, op1=ALU.add)
                    nc.sync.dma_start(
                        out=out_t[b, :, w0:gcols[-1] + 1:w1 - w0, :],
                        in_=ev[:, :, :])
    nc.compile()
    return nc


def kernel(x, cond, alpha_w, alpha_b, expert_kernels, expert_bias, trace=False):
    if "nc" not in _cache:
        _cache["nc"] = _build_nc()
    nc = _cache["nc"]
    aw = np.ascontiguousarray(np.asarray(alpha_w, dtype=np.float32))
    ab = np.ascontiguousarray(np.asarray(alpha_b, dtype=np.float32))
    ek = np.ascontiguousarray(np.asarray(expert_kernels, dtype=np.float32))
    eb = np.ascontiguousarray(np.asarray(expert_bias, dtype=np.float32))
    x = np.asarray(x, dtype=np.float32)
    cond = np.asarray(cond, dtype=np.float32)
    in_maps = []
    for c in range(NCORES):
        in_maps.append({
            "x": np.ascontiguousarray(x[c * NB:(c + 1) * NB]),
            "cond": np.ascontiguousarray(cond[c * NB:(c + 1) * NB]),
            "alpha_w": aw, "alpha_b": ab,
            "expert_kernels": ek, "expert_bias": eb,
        })
    res = run_bass_kernel_spmd(nc, in_maps, core_ids=list(range(NCORES)), trace=trace)
    _cache["last_result"] = res
    return np.concatenate([r["out"] for r in res.results], axis=0)
